# revision 7
# baseline (speedup 1.0000x reference)
"""CrossWinAttention Trainium2 kernel.

Data-parallel over the 128 (batch, window) pairs: 16 per NeuronCore x 8 cores.
Each core runs the full LN + QKV projection + 4-head attention + output
projection + view-mean + skip-add for its pairs.

Device-side layout strategy (per (b,l) pair, Q=384 tokens, D=HD=128):
  - token-major LN (bn_stats/bn_aggr, rsqrt via ln/exp on ACT)
  - PE-transpose xhat -> feature-major xhatT
  - q,k projections feature-major (out [HD,384]); v projection token-major
  - scoresT[k,q] per head via 32-row tile-packed matmuls (4 heads concurrent)
  - exp on ACT straight out of PSUM (no max-subtract; |scores| ~ 8)
  - AV matmul with a fused "sixes" column producing 6*Z (softmax denom * 6,
    folding the 1/6 view-mean) as an extra output row; M=33, col-tiled 2/bank
  - softmax normalization deferred: av * (1/(6Z)) with 1/(6Z) broadcast
    across partitions by a tiny block-indicator matmul on the PE
  - view-mean as a strided reduce, output projection on [HD,64], skip add
Biases: LN gamma, q-scale, lnb/b biases are folded on the host (q-side bias
is exactly softmax-invariant; its k-dependent cross term is emitted as extra
accumulate matmuls only when nonzero).
"""

import numpy as np
from contextlib import ExitStack

import concourse.bass as bass
import concourse.tile as tile
from concourse import bacc, mybir
from concourse.bass_utils import run_bass_kernel_spmd

# Problem dims (hardcoded per contest rules).
B, N, X, Y, W1, W2, D = 2, 6, 8, 8, 8, 8, 128
H, DH = 4, 32
HD = H * DH
L = X * Y                  # 64 windows
Q = N * W1 * W2            # 384 tokens per window
BL = B * L                 # 128 (b,l) pairs
NCORES = 8
PER_CORE = BL // NCORES    # 16
NW = W1 * W2               # 64
EPS = 1e-5
F32 = mybir.dt.float32

_COMPILED = {}


def _emit(nc, with_qbias_cross):
    f32 = F32
    din = {}
    for t in ("xq", "xk", "xv"):
        din[t] = nc.dram_tensor(t, [PER_CORE, Q, D], f32, kind="ExternalInput").ap()
    skip_d = nc.dram_tensor("skipT", [PER_CORE, D, NW], f32, kind="ExternalInput").ap()
    wq_d = nc.dram_tensor("wq", [D, HD], f32, kind="ExternalInput").ap()
    wk_d = nc.dram_tensor("wk", [D, HD], f32, kind="ExternalInput").ap()
    wv_d = nc.dram_tensor("wv", [D, HD], f32, kind="ExternalInput").ap()
    wpa_d = nc.dram_tensor("wpA", [128, D], f32, kind="ExternalInput").ap()
    wpb_d = nc.dram_tensor("wpB", [128, D], f32, kind="ExternalInput").ap()
    bk_d = nc.dram_tensor("bk", [HD, 1], f32, kind="ExternalInput").ap()
    ea_d = nc.dram_tensor("EbA", [H, 128], f32, kind="ExternalInput").ap()
    eb_d = nc.dram_tensor("EbB", [H, 128], f32, kind="ExternalInput").ap()
    id_d = nc.dram_tensor("ident", [128, 128], f32, kind="ExternalInput").ap()
    # feature-major q-bias cross row (c = bqp^T kpT per head), built on device
    # from U = wk' @ diag-blocks... passed as [D, H] projection matrix instead.
    u_d = nc.dram_tensor("U", [D, H], f32, kind="ExternalInput").ap()
    g_d = nc.dram_tensor("gam", [H, 1], f32, kind="ExternalInput").ap()
    out_d = nc.dram_tensor("out", [PER_CORE, D, NW], f32, kind="ExternalOutput").ap()

    Exp = mybir.ActivationFunctionType.Exp
    Ln = mybir.ActivationFunctionType.Ln
    Alu = mybir.AluOpType

    with tile.TileContext(nc) as tc, ExitStack() as ctx:
        const = ctx.enter_context(tc.tile_pool(name="const", bufs=1))
        sb = ctx.enter_context(tc.tile_pool(name="sb", bufs=2))
        attp = ctx.enter_context(tc.tile_pool(name="attp", bufs=3))
        # PSUM: pa(2) + sc(4) + av(2) = 8 banks exactly
        pa = ctx.enter_context(tc.tile_pool(name="pa", bufs=2, space="PSUM"))
        scp = ctx.enter_context(tc.tile_pool(name="scp", bufs=1, space="PSUM"))
        avp = ctx.enter_context(tc.tile_pool(name="avp", bufs=1, space="PSUM"))

        def cload(name, ap_, shape):
            t = const.tile(shape, f32, tag=name, name=name)
            nc.sync.dma_start(t[:], ap_[:])
            return t

        wq_sb = cload("wq", wq_d, [D, HD])
        wk_sb = cload("wk", wk_d, [D, HD])
        wv_sb = cload("wv", wv_d, [D, HD])
        wpa_sb = cload("wpA", wpa_d, [128, D])
        wpb_sb = cload("wpB", wpb_d, [128, D])
        bk_sb = cload("bk", bk_d, [HD, 1])
        ea_sb = cload("EbA", ea_d, [H, 128])
        eb_sb = cload("EbB", eb_d, [H, 128])
        id_sb = cload("ident", id_d, [128, 128])
        eps_sb = const.tile([128, 1], f32, tag="eps", name="eps")
        nc.vector.memset(eps_sb[:], EPS)
        if with_qbias_cross:
            u_sb = cload("U", u_d, [D, H])
            g_sb = cload("gam", g_d, [H, 1])
            ones_sb = const.tile([1, Q], f32, tag="ones")
            nc.vector.memset(ones_sb[:], 1.0)

        for bl in range(PER_CORE):
            # ---- load token-major x tiles [128 tok, 3 chunks, 128 D]
            x_sb = {}
            for t in ("xq", "xk", "xv"):
                x_sb[t] = sb.tile([128, 3, D], f32, tag=t, name=t)
                nc.sync.dma_start(
                    x_sb[t][:], din[t][bl].rearrange("(c p) d -> p c d", p=128)
                )
            skip_sb = sb.tile([D, NW], f32, tag="skip")
            nc.sync.dma_start(skip_sb[:], skip_d[bl])

            # ---- LN stats (token-major): mean/var per token, r=1/sqrt(var+eps)
            st = sb.tile([128, 3, 3, 2], f32, tag="st")   # [tok, chunk, tensor, (mu,var)]
            for c in range(3):
                for ti, t in enumerate(("xq", "xk", "xv")):
                    bn6 = sb.tile([128, 6], f32, tag="bn6")
                    nc.vector.bn_stats(bn6[:], x_sb[t][:, c, :])
                    nc.vector.bn_aggr(st[:, c, ti, :], bn6[:])
            r9 = sb.tile([128, 3, 3], f32, tag="r9")      # rsqrt(var+eps)
            nc.scalar.activation(r9[:], st[:, :, :, 1], Ln, bias=eps_sb[:])
            nc.scalar.activation(r9[:], r9[:], Exp, scale=-0.5)

            # ---- xhat = (x - mu) * r (gpsimd), then PE-transpose to xhatT
            xhT_sb = {}
            for ti, t in enumerate(("xq", "xk", "xv")):
                xh = sb.tile([128, 3, D], f32, tag=f"xh{t}")
                for c in range(3):
                    nc.gpsimd.tensor_scalar(
                        xh[:, c, :], x_sb[t][:, c, :],
                        st[:, c, ti, 0:1], r9[:, c, ti : ti + 1],
                        op0=Alu.subtract, op1=Alu.mult,
                    )
                xhT_ps = pa.tile([128, 512], f32, tag="pa")
                for c in range(3):
                    nc.tensor.transpose(
                        xhT_ps[:, 128 * c : 128 * (c + 1)], xh[:, c, :], id_sb[:]
                    )
                xhT_sb[t] = sb.tile([D, Q], f32, tag=f"xhT{t}", name=f"xhT{t}")
                nc.vector.tensor_copy(xhT_sb[t][:], xhT_ps[:, 0:Q])

            # ---- projections: q,k feature-major [HD, 384]; v token-major
            qpT_sb = sb.tile([HD, Q], f32, tag="qpT")
            kpT_sb = sb.tile([HD, Q], f32, tag="kpT")
            for wsb, xsb, osb, bias in (
                (wq_sb, xhT_sb["xq"], qpT_sb, None),
                (wk_sb, xhT_sb["xk"], kpT_sb, bk_sb),
            ):
                pp = pa.tile([128, 512], f32, tag="pa")
                nc.tensor.matmul(pp[:, 0:Q], wsb[:], xsb[:])
                if bias is None:
                    nc.vector.tensor_copy(osb[:], pp[:, 0:Q])
                else:
                    nc.vector.tensor_scalar(
                        osb[:], pp[:, 0:Q], bias[:], None, op0=Alu.add
                    )
            if with_qbias_cross:
                # kaugT[h, k] = xhat_k . U_h + gamma_h, feature-major row per head
                ka_ps = pa.tile([128, 512], f32, tag="pa")
                nc.tensor.matmul(ka_ps[0:H, 0:Q], u_sb[:], xhT_sb["xk"][:])
                ka_sb = sb.tile([H, Q], f32, tag="ka")
                nc.vector.tensor_scalar(
                    ka_sb[:], ka_ps[0:H, 0:Q], g_sb[0:H, :], None, op0=Alu.add
                )

            vp_ps = pa.tile([128, 512], f32, tag="pa")
            for c in range(3):
                nc.tensor.matmul(
                    vp_ps[:, 128 * c : 128 * (c + 1)],
                    xhT_sb["xv"][:, 128 * c : 128 * (c + 1)], wv_sb[:],
                )
            # vpe: per chunk, per head: [v_h (32 cols) | 6.0] -> [128, 3, 132]
            vpe = sb.tile([128, 3, H * (DH + 1)], f32, tag="vpe")
            for c in range(3):
                vv = vpe[:, c, :].rearrange("p (h w) -> p h w", w=DH + 1)
                nc.vector.tensor_copy(
                    vv[:, :, 0:DH],
                    vp_ps[:, 128 * c : 128 * (c + 1)].rearrange(
                        "p (h w) -> p h w", w=DH
                    ),
                )
                nc.gpsimd.memset(vv[:, :, DH : DH + 1], 6.0)

            # ---- attention: scoresT + exp per k-chunk, then AV per head
            av_ps = avp.tile([128, 1024], f32, tag="av")
            atts = []
            for c in range(3):
                sc_ps = scp.tile([128, 2048], f32, tag="sc")
                for h in range(H):
                    nc.tensor.matmul(
                        sc_ps[:, 512 * h : 512 * h + Q],
                        kpT_sb[32 * h : 32 * (h + 1), 128 * c : 128 * (c + 1)],
                        qpT_sb[32 * h : 32 * (h + 1), :],
                        tile_position=(32 * h, 0),
                        start=True, stop=not with_qbias_cross,
                    )
                    if with_qbias_cross:
                        nc.tensor.matmul(
                            sc_ps[:, 512 * h : 512 * h + Q],
                            ka_sb[h : h + 1, 128 * c : 128 * (c + 1)],
                            ones_sb[:],
                            start=False, stop=True,
                        )
                att = attp.tile([128, H, Q], f32, tag="att", name=f"att{c}")
                nc.scalar.activation(
                    att[:],
                    sc_ps[:].rearrange("p (h q) -> p h q", q=512)[:, :, 0:Q],
                    Exp,
                )
                atts.append(att)
            for h in range(H):
                po, bo = 64 * (h % 2), 512 * (h // 2)
                for c in range(3):
                    nc.tensor.matmul(
                        av_ps[po : po + DH + 1, bo : bo + Q],
                        vpe[:, c, 33 * h : 33 * (h + 1)],
                        atts[c][:, h, :],
                        tile_position=(0, po),
                        start=(c == 0), stop=(c == 2),
                    )

            # ---- normalize by 1/(6Z), mean over views, project, add skip
            # av_ps native layout: bank b (cols 512b), head 2b at partitions
            # 0:32 (Z at 32), head 2b+1 at partitions 64:96 (Z at 96).
            av_sb = sb.tile([128, 2, Q], f32, tag="av_sb")
            nc.gpsimd.memset(av_sb[32:64, :, :], 0.0)
            nc.gpsimd.memset(av_sb[96:128, :, :], 0.0)
            for h in range(H):
                po, bo = 64 * (h % 2), 512 * (h // 2)
                nc.vector.tensor_copy(
                    av_sb[po : po + DH + 1, h // 2, :],
                    av_ps[po : po + DH + 1, bo : bo + Q],
                )
            zrow = sb.tile([H, Q], f32, tag="zrow")
            for h in range(H):
                po = 64 * (h % 2) + DH
                nc.sync.dma_start(
                    zrow[h : h + 1, :], av_sb[po : po + 1, h // 2, :]
                )
            zinv = sb.tile([H, Q], f32, tag="zinv")
            nc.vector.reciprocal(zinv[:], zrow[:])
            zb_ps = avp.tile([128, 1024], f32, tag="av", name="zb_ps")
            nc.tensor.matmul(zb_ps[:, 0:Q], ea_sb[:], zinv[:])
            nc.tensor.matmul(zb_ps[:, 512 : 512 + Q], eb_sb[:], zinv[:])
            avn = sb.tile([128, 2, Q], f32, tag="avn")
            nc.vector.tensor_tensor(
                avn[:], av_sb[:],
                zb_ps[:].rearrange("p (b q) -> p b q", q=512)[:, :, 0:Q],
                op=Alu.mult,
            )
            avm = sb.tile([128, 2, NW], f32, tag="avm")
            nc.vector.reduce_sum(
                avm[:], avn[:].rearrange("p b (n w) -> p b w n", n=N),
                axis=mybir.AxisListType.X,
            )
            z_ps = pa.tile([128, 512], f32, tag="pa", name="z_ps")
            nc.tensor.matmul(z_ps[:, 0:NW], wpa_sb[:], avm[:, 0, :], start=True, stop=False)
            nc.tensor.matmul(z_ps[:, 0:NW], wpb_sb[:], avm[:, 1, :], start=False, stop=True)
            zo = sb.tile([D, NW], f32, tag="zo")
            nc.vector.tensor_tensor(zo[:], z_ps[:, 0:NW], skip_sb[:], op=Alu.add)
            nc.sync.dma_start(out_d[bl], zo[:])

def _build(with_qbias_cross):
    key = bool(with_qbias_cross)
    if key in _COMPILED:
        return _COMPILED[key]
    nc = bacc.Bacc("TRN2", target_bir_lowering=False, debug=False)
    _emit(nc, key)
    nc.compile()
    _COMPILED[key] = nc
    return nc


def _prep_host(inputs):
    q, k, v, skip = inputs["q"], inputs["k"], inputs["v"], inputs["skip"]
    scale = np.float32(DH ** -0.5)
    fold = lambda t: np.ascontiguousarray(
        t.transpose(0, 2, 3, 1, 4, 5, 6).reshape(BL, Q, D)
    )
    xq, xk, xv = fold(q), fold(k), fold(v)
    wq = (inputs["lnq_g"][:, None] * inputs["wq"] * scale).astype(np.float32)
    wk = (inputs["lnk_g"][:, None] * inputs["wk"]).astype(np.float32)
    wv = (inputs["lnv_g"][:, None] * inputs["wv"]).astype(np.float32)
    wp = inputs["wp"].astype(np.float32)
    bkp = (inputs["lnk_b"] @ inputs["wk"] + inputs["bk"]).astype(np.float32)
    bqp = ((inputs["lnq_b"] @ inputs["wq"] + inputs["bq"]) * scale).astype(np.float32)
    bvp = (inputs["lnv_b"] @ inputs["wv"] + inputs["bv"]).astype(np.float32)
    skipT = np.ascontiguousarray(
        (skip.reshape(BL, NW, D) + inputs["bp"] + bvp @ wp).transpose(0, 2, 1)
    ).astype(np.float32)
    # q-side bias: softmax-invariant part drops; k-dependent cross term needs
    # U[:, h] = wk'_hblock @ bqp_hblock and gamma_h = bk'_h . bqp_h
    U = np.zeros((D, H), np.float32)
    gam = np.zeros((H, 1), np.float32)
    for h in range(H):
        s = slice(h * DH, (h + 1) * DH)
        U[:, h] = wk[:, s] @ bqp[s]
        gam[h, 0] = bkp[s] @ bqp[s]
    with_cross = bool(np.abs(bqp).max() > 0)
    # native AV layout remap: bank A heads 0,1; bank B heads 2,3;
    # head pair member m at partitions 64m..64m+32 (Z row at 64m+32)
    wpA = np.zeros((128, D), np.float32)
    wpB = np.zeros((128, D), np.float32)
    EbA = np.zeros((H, 128), np.float32)
    EbB = np.zeros((H, 128), np.float32)
    for m in range(2):
        po = 64 * m
        wpA[po : po + DH] = wp[m * DH : (m + 1) * DH]
        wpB[po : po + DH] = wp[(2 + m) * DH : (3 + m) * DH]
        EbA[m, po : po + DH] = 1.0
        EbB[2 + m, po : po + DH] = 1.0
    ident = np.eye(128, dtype=np.float32)
    consts = dict(
        wq=wq, wk=wk, wv=wv, wpA=wpA, wpB=wpB, bk=bkp.reshape(HD, 1),
        EbA=EbA, EbB=EbB, ident=ident, U=U, gam=gam,
    )
    in_maps = []
    for c in range(NCORES):
        s = slice(c * PER_CORE, (c + 1) * PER_CORE)
        m = dict(
            xq=np.ascontiguousarray(xq[s]),
            xk=np.ascontiguousarray(xk[s]),
            xv=np.ascontiguousarray(xv[s]),
            skipT=np.ascontiguousarray(skipT[s]),
        )
        m.update({k_: v_.copy() for k_, v_ in consts.items()})
        in_maps.append(m)
    return in_maps, with_cross


def kernel(**inputs):
    inputs = {k: np.asarray(v, dtype=np.float32) for k, v in inputs.items()}
    in_maps, with_cross = _prep_host(inputs)
    nc = _build(with_cross)
    res = run_bass_kernel_spmd(nc, in_maps, list(range(NCORES)))
    zT = np.concatenate([r["out"] for r in res.results], axis=0)  # [BL, D, 64]
    z = zT.transpose(0, 2, 1).reshape(B, X, Y, W1, W2, D)
    return np.ascontiguousarray(z)


# revision 22
# speedup vs baseline: 1.0922x; 1.0922x over previous
"""CrossWinAttention Trainium2 kernel.

Data-parallel over the 128 (batch, window) pairs: 16 per NeuronCore x 8 cores.
Each core runs the full LN + QKV projection + 4-head attention + output
projection + view-mean + skip-add for its pairs.

Device-side layout strategy (per (b,l) pair, Q=384 tokens, D=HD=128):
  - token-major LN (bn_stats/bn_aggr, rsqrt via ln/exp on ACT)
  - PE-transpose xhat -> feature-major xhatT
  - q,k projections feature-major (out [HD,384]); v projection token-major
  - scoresT[k,q] per head via 32-row tile-packed matmuls (4 heads concurrent)
  - exp on ACT straight out of PSUM (no max-subtract; |scores| ~ 8)
  - AV matmul with a fused "sixes" column producing 6*Z (softmax denom * 6,
    folding the 1/6 view-mean) as an extra output row; M=33, col-tiled 2/bank
  - softmax normalization deferred: av * (1/(6Z)) with 1/(6Z) broadcast
    across partitions by a tiny block-indicator matmul on the PE
  - view-mean as a strided reduce, output projection on [HD,64], skip add
Biases: LN gamma, q-scale, lnb/b biases are folded on the host (q-side bias
is exactly softmax-invariant; its k-dependent cross term is emitted as extra
accumulate matmuls only when nonzero).
"""

import numpy as np
from contextlib import ExitStack

import concourse.bass as bass
import concourse.tile as tile
from concourse import bacc, mybir
from concourse.bass_utils import run_bass_kernel_spmd

# Problem dims (hardcoded per contest rules).
B, N, X, Y, W1, W2, D = 2, 6, 8, 8, 8, 8, 128
H, DH = 4, 32
HD = H * DH
L = X * Y                  # 64 windows
Q = N * W1 * W2            # 384 tokens per window
BL = B * L                 # 128 (b,l) pairs
NCORES = 8
PER_CORE = BL // NCORES    # 16
NW = W1 * W2               # 64
EPS = 1e-5
F32 = mybir.dt.float32

_COMPILED = {}
USE_FP32R = True


def _emit(nc, with_qbias_cross):
    f32 = F32
    din = {}
    for t in ("xq", "xk", "xv"):
        din[t] = nc.dram_tensor(t, [PER_CORE, Q, D], f32, kind="ExternalInput").ap()
    skip_d = nc.dram_tensor("skipT", [PER_CORE, D, NW], f32, kind="ExternalInput").ap()
    _wdt = mybir.dt.float32r if USE_FP32R else f32
    wq_d = nc.dram_tensor("wq", [D, HD], _wdt, kind="ExternalInput").ap()
    wk_d = nc.dram_tensor("wk", [D, HD], _wdt, kind="ExternalInput").ap()
    wv_d = nc.dram_tensor("wv", [D, HD], _wdt, kind="ExternalInput").ap()
    wpa_d = nc.dram_tensor("wpA", [128, D], f32, kind="ExternalInput").ap()
    wpb_d = nc.dram_tensor("wpB", [128, D], f32, kind="ExternalInput").ap()
    bk_d = nc.dram_tensor("bk", [HD, 1], f32, kind="ExternalInput").ap()
    ea_d = nc.dram_tensor("EbA", [H, 128], f32, kind="ExternalInput").ap()
    eb_d = nc.dram_tensor("EbB", [H, 128], f32, kind="ExternalInput").ap()
    id_d = nc.dram_tensor("ident", [128, 128], f32, kind="ExternalInput").ap()
    # feature-major q-bias cross row (c = bqp^T kpT per head), built on device
    # from U = wk' @ diag-blocks... passed as [D, H] projection matrix instead.
    u_d = nc.dram_tensor("U", [D, H], f32, kind="ExternalInput").ap()
    g_d = nc.dram_tensor("gam", [H, 1], f32, kind="ExternalInput").ap()
    out_d = nc.dram_tensor("out", [PER_CORE, D, NW], f32, kind="ExternalOutput").ap()

    if USE_FP32R:
        r32 = lambda ap: ap.bitcast(mybir.dt.float32r)
        fr = mybir.dt.float32r
    else:
        r32 = lambda ap: ap
        fr = f32
    Exp = mybir.ActivationFunctionType.Exp
    Ln = mybir.ActivationFunctionType.Ln
    Alu = mybir.AluOpType

    with tile.TileContext(nc) as tc, ExitStack() as ctx:
        const = ctx.enter_context(tc.tile_pool(name="const", bufs=1))
        sb = ctx.enter_context(tc.tile_pool(name="sb", bufs=2))
        attp = ctx.enter_context(tc.tile_pool(name="attp", bufs=3))
        # PSUM: pa(2) + sc(4) + av(2) = 8 banks exactly
        pa = ctx.enter_context(tc.tile_pool(name="pa", bufs=2, space="PSUM"))
        scp = ctx.enter_context(tc.tile_pool(name="scp", bufs=1, space="PSUM"))
        avp = ctx.enter_context(tc.tile_pool(name="avp", bufs=1, space="PSUM"))
        dramp = ctx.enter_context(tc.tile_pool(name="dramp", bufs=2, space="DRAM"))

        def cload(name, ap_, shape, dt_=f32):
            t = const.tile(shape, dt_, tag=name, name=name)
            nc.sync.dma_start(t[:], ap_[:])
            return t

        wq_sb = cload("wq", wq_d, [D, HD], fr)
        wk_sb = cload("wk", wk_d, [D, HD], fr)
        wv_sb = cload("wv", wv_d, [D, HD], fr)
        wpa_sb = cload("wpA", wpa_d, [128, D])
        wpb_sb = cload("wpB", wpb_d, [128, D])
        bk_sb = cload("bk", bk_d, [HD, 1])
        ea_sb = cload("EbA", ea_d, [H, 128])
        eb_sb = cload("EbB", eb_d, [H, 128])
        id_sb = cload("ident", id_d, [128, 128])
        eps_sb = const.tile([128, 1], f32, tag="eps", name="eps")
        nc.vector.memset(eps_sb[:], EPS)
        if with_qbias_cross:
            u_sb = cload("U", u_d, [D, H])
            g_sb = cload("gam", g_d, [H, 1])
            ones_sb = const.tile([1, Q], fr, tag="ones")
            nc.vector.memset(ones_sb[:], 1.0)

        for bl in range(PER_CORE):
            # ---- load token-major x tiles [128 tok, 3 chunks, 128 D]
            x_sb = {}
            for t in ("xq", "xk", "xv"):
                x_sb[t] = sb.tile([128, 3, D], f32, tag=t, name=t)
                nc.sync.dma_start(
                    x_sb[t][:], din[t][bl].rearrange("(c p) d -> p c d", p=128)
                )
            skip_sb = sb.tile([D, NW], f32, tag="skip")
            nc.sync.dma_start(skip_sb[:], skip_d[bl])

            # ---- LN stats (token-major): mean/var per token, r=1/sqrt(var+eps)
            st = sb.tile([128, 3, 3, 2], f32, tag="st")   # [tok, chunk, tensor, (mu,var)]
            for ti, t in enumerate(("xq", "xk", "xv")):
                bn6 = sb.tile([128, 3, 6], f32, tag="bn6")
                for c in range(3):
                    nc.vector.bn_stats(bn6[:, c, :], x_sb[t][:, c, :])
                    nc.vector.bn_aggr(st[:, c, ti, :], bn6[:, c, :])
            r9 = sb.tile([128, 3, 3], f32, tag="r9")      # rsqrt(var+eps)
            nc.scalar.activation(r9[:], st[:, :, :, 1], Ln, bias=eps_sb[:])
            nc.scalar.activation(r9[:], r9[:], Exp, scale=-0.5)

            # ---- xhat = (x - mu) * r (gpsimd), then PE-transpose to xhatT
            xhT_sb = {}
            for ti, t in enumerate(("xq", "xk", "xv")):
                xh = sb.tile([128, 3, D], f32, tag=f"xh{t}")
                for c in range(3):
                    nc.gpsimd.tensor_scalar(
                        xh[:, c, :], x_sb[t][:, c, :],
                        st[:, c, ti, 0:1], r9[:, c, ti : ti + 1],
                        op0=Alu.subtract, op1=Alu.mult,
                    )
                xhT_ps = pa.tile([128, 512], f32, tag="pa")
                for c in range(3):
                    nc.tensor.transpose(
                        xhT_ps[:, 128 * c : 128 * (c + 1)], xh[:, c, :], id_sb[:]
                    )
                xhT_sb[t] = sb.tile([D, Q], fr, tag=f"xhT{t}", name=f"xhT{t}")
                nc.vector.tensor_copy(xhT_sb[t][:], xhT_ps[:, 0:Q])

            # ---- projections: q,k feature-major [HD, 384]; v token-major
            qpT_sb = sb.tile([HD, Q], fr, tag="qpT")
            kpT_sb = sb.tile([HD, Q], fr, tag="kpT")
            for wsb, xsb, osb, bias in (
                (wq_sb, xhT_sb["xq"], qpT_sb, None),
                (wk_sb, xhT_sb["xk"], kpT_sb, bk_sb),
            ):
                pp = pa.tile([128, 512], f32, tag="pa")
                nc.tensor.matmul(pp[:, 0:Q], r32(wsb[:]), r32(xsb[:]))
                if bias is None:
                    nc.scalar.copy(osb[:], pp[:, 0:Q])
                else:
                    nc.vector.tensor_scalar(
                        osb[:], pp[:, 0:Q], bias[:], None, op0=Alu.add
                    )
            if with_qbias_cross:
                # kaugT[h, k] = xhat_k . U_h + gamma_h, feature-major row per head
                ka_ps = pa.tile([128, 512], f32, tag="pa")
                nc.tensor.matmul(ka_ps[0:H, 0:Q], u_sb[:], xhT_sb["xk"][:])
                ka_sb = sb.tile([H, Q], fr, tag="ka")
                nc.vector.tensor_scalar(
                    ka_sb[:], ka_ps[0:H, 0:Q], g_sb[0:H, :], None, op0=Alu.add
                )

            vp_ps = pa.tile([128, 512], f32, tag="pa")
            for c in range(3):
                nc.tensor.matmul(
                    vp_ps[:, 128 * c : 128 * (c + 1)],
                    xhT_sb["xv"][:, 128 * c : 128 * (c + 1)], wv_sb[:],
                )
            # vpe: per chunk, per head: [v_h (32 cols) | 6.0] -> [128, 3, 132]
            vpe = sb.tile([128, 3, H * (DH + 1)], mybir.dt.bfloat16, tag="vpe")
            for c in range(3):
                vv = vpe[:, c, :].rearrange("p (h w) -> p h w", w=DH + 1)
                nc.vector.tensor_copy(
                    vv[:, :, 0:DH],
                    vp_ps[:, 128 * c : 128 * (c + 1)].rearrange(
                        "p (h w) -> p h w", w=DH
                    ),
                )
                nc.gpsimd.memset(vv[:, :, DH : DH + 1], 6.0)

            # ---- attention: scoresT + exp per k-chunk, then AV per head
            av_ps = avp.tile([128, 1024], f32, tag="av")
            atts = []
            for c in range(3):
                sc_ps = scp.tile([128, 2048], f32, tag="sc")
                for h in range(H):
                    nc.tensor.matmul(
                        sc_ps[:, 512 * h : 512 * h + Q],
                        r32(kpT_sb[32 * h : 32 * (h + 1), 128 * c : 128 * (c + 1)]),
                        r32(qpT_sb[32 * h : 32 * (h + 1), :]),
                        tile_position=(32 * h, 0),
                        start=True, stop=not with_qbias_cross,
                    )
                    if with_qbias_cross:
                        nc.tensor.matmul(
                            sc_ps[:, 512 * h : 512 * h + Q],
                            r32(ka_sb[h : h + 1, 128 * c : 128 * (c + 1)]),
                            r32(ones_sb[:]),
                            start=False, stop=True,
                        )
                att = attp.tile([128, H, Q], mybir.dt.bfloat16, tag="att", name=f"att{c}")
                nc.scalar.activation(
                    att[:],
                    sc_ps[:].rearrange("p (h q) -> p h q", q=512)[:, :, 0:Q],
                    Exp,
                )
                atts.append(att)
            for h in range(H):
                po, bo = 64 * (h % 2), 512 * (h // 2)
                for c in range(3):
                    nc.tensor.matmul(
                        av_ps[po : po + DH + 1, bo : bo + Q],
                        vpe[:, c, 33 * h : 33 * (h + 1)],
                        atts[c][:, h, :],
                        tile_position=(0, po),
                        start=(c == 0), stop=(c == 2),
                    )

            # ---- normalize by 1/(6Z), mean over views, project, add skip
            # av_ps native layout: bank b (cols 512b), head-pair member m at
            # partitions 64m:64m+32, its 6Z row at partition 64m+32.
            zi_sb = sb.tile([128, 2, Q], f32, tag="zi_sb")
            for p2 in range(2):
                po = 32 + 64 * p2
                nc.vector.reciprocal(
                    zi_sb[po : po + 1],
                    av_ps[po : po + 1].rearrange("p (b q) -> p b q", q=512)[:, :, 0:Q],
                )
            zr = [sb.tile([1, Q], f32, tag=f"zr{r}", name=f"zr{r}") for r in range(H)]
            for p2 in range(2):
                for b in range(2):
                    nc.sync.dma_start(
                        zr[2 * p2 + b][:], zi_sb[32 + 64 * p2 : 33 + 64 * p2, b, :]
                    )
            zdram = dramp.tile([H, Q], f32, tag="zdram", name="zdram")
            for r in range(H):
                nc.sync.dma_start(zdram[r : r + 1, :], zr[r][:])
            zbb = sb.tile([128, 2, Q], f32, tag="zbb")
            for po in (0, 64):
                for b in range(2):
                    r = 2 * (po // 64) + b
                    nc.sync.dma_start(
                        zbb[po : po + DH, b, :],
                        zdram[r : r + 1, :].partition_broadcast(DH),
                    )
            avn = sb.tile([128, 2, Q], f32, tag="avn")
            nc.gpsimd.memset(avn[32:64, :, :], 0.0)
            nc.gpsimd.memset(avn[96:128, :, :], 0.0)
            for po in (0, 64):
                nc.vector.tensor_tensor(
                    avn[po : po + DH, :, :],
                    av_ps[po : po + DH].rearrange("p (b q) -> p b q", q=512)[:, :, 0:Q],
                    zbb[po : po + DH, :, :],
                    op=Alu.mult,
                )
            avm = sb.tile([128, 2, NW], f32, tag="avm")
            nc.vector.reduce_sum(
                avm[:], avn[:].rearrange("p b (n w) -> p b w n", n=N),
                axis=mybir.AxisListType.X,
            )
            z_ps = pa.tile([128, 512], f32, tag="pa", name="z_ps")
            nc.tensor.matmul(z_ps[:, 0:NW], wpa_sb[:], avm[:, 0, :], start=True, stop=False)
            nc.tensor.matmul(z_ps[:, 0:NW], wpb_sb[:], avm[:, 1, :], start=False, stop=True)
            zo = sb.tile([D, NW], f32, tag="zo")
            nc.vector.tensor_tensor(zo[:], z_ps[:, 0:NW], skip_sb[:], op=Alu.add)
            nc.sync.dma_start(out_d[bl], zo[:])


def _build(with_qbias_cross):
    key = (bool(with_qbias_cross), USE_FP32R)
    if key in _COMPILED:
        return _COMPILED[key]
    nc = bacc.Bacc("TRN2", target_bir_lowering=False, debug=False)
    _emit(nc, bool(with_qbias_cross))
    nc.compile()
    _COMPILED[key] = nc
    return nc


def _prep_host(inputs):
    q, k, v, skip = inputs["q"], inputs["k"], inputs["v"], inputs["skip"]
    scale = np.float32(DH ** -0.5)
    fold = lambda t: np.ascontiguousarray(
        t.transpose(0, 2, 3, 1, 4, 5, 6).reshape(BL, Q, D)
    )
    xq, xk, xv = fold(q), fold(k), fold(v)
    wq = (inputs["lnq_g"][:, None] * inputs["wq"] * scale).astype(np.float32)
    wk = (inputs["lnk_g"][:, None] * inputs["wk"]).astype(np.float32)
    wv = (inputs["lnv_g"][:, None] * inputs["wv"]).astype(np.float32)
    wp = inputs["wp"].astype(np.float32)
    bkp = (inputs["lnk_b"] @ inputs["wk"] + inputs["bk"]).astype(np.float32)
    bqp = ((inputs["lnq_b"] @ inputs["wq"] + inputs["bq"]) * scale).astype(np.float32)
    bvp = (inputs["lnv_b"] @ inputs["wv"] + inputs["bv"]).astype(np.float32)
    skipT = np.ascontiguousarray(
        (skip.reshape(BL, NW, D) + inputs["bp"] + bvp @ wp).transpose(0, 2, 1)
    ).astype(np.float32)
    # q-side bias: softmax-invariant part drops; k-dependent cross term needs
    # U[:, h] = wk'_hblock @ bqp_hblock and gamma_h = bk'_h . bqp_h
    U = np.zeros((D, H), np.float32)
    gam = np.zeros((H, 1), np.float32)
    for h in range(H):
        s = slice(h * DH, (h + 1) * DH)
        U[:, h] = wk[:, s] @ bqp[s]
        gam[h, 0] = bkp[s] @ bqp[s]
    with_cross = bool(np.abs(bqp).max() > 0)
    # native AV layout remap: bank A heads 0,1; bank B heads 2,3;
    # head pair member m at partitions 64m..64m+32 (Z row at 64m+32)
    wpA = np.zeros((128, D), np.float32)
    wpB = np.zeros((128, D), np.float32)
    EbA = np.zeros((H, 128), np.float32)
    EbB = np.zeros((H, 128), np.float32)
    for m in range(2):
        po = 64 * m
        wpA[po : po + DH] = wp[m * DH : (m + 1) * DH]
        wpB[po : po + DH] = wp[(2 + m) * DH : (3 + m) * DH]
        # zrow row r = 2*p2 + b holds (6Z) of head h = 2b + p2; here p2 = m.
        EbA[2 * m + 0, po : po + DH] = 1.0   # bank A (b=0): heads 0,1
        EbB[2 * m + 1, po : po + DH] = 1.0   # bank B (b=1): heads 2,3
    ident = np.eye(128, dtype=np.float32)
    consts = dict(
        wq=wq, wk=wk, wv=wv, wpA=wpA, wpB=wpB, bk=bkp.reshape(HD, 1),
        EbA=EbA, EbB=EbB, ident=ident, U=U, gam=gam,
    )
    in_maps = []
    for c in range(NCORES):
        s = slice(c * PER_CORE, (c + 1) * PER_CORE)
        m = dict(
            xq=np.ascontiguousarray(xq[s]),
            xk=np.ascontiguousarray(xk[s]),
            xv=np.ascontiguousarray(xv[s]),
            skipT=np.ascontiguousarray(skipT[s]),
        )
        m.update({k_: v_.copy() for k_, v_ in consts.items()})
        in_maps.append(m)
    return in_maps, with_cross


def kernel(**inputs):
    inputs = {k: np.asarray(v, dtype=np.float32) for k, v in inputs.items()}
    in_maps, with_cross = _prep_host(inputs)
    nc = _build(with_cross)
    res = run_bass_kernel_spmd(nc, in_maps, list(range(NCORES)))
    zT = np.concatenate([r["out"] for r in res.results], axis=0)  # [BL, D, 64]
    z = zT.transpose(0, 2, 1).reshape(B, X, Y, W1, W2, D)
    return np.ascontiguousarray(z)


# revision 27
# speedup vs baseline: 1.7576x; 1.6093x over previous
"""CrossWinAttention Trainium2 kernel.

Data-parallel over the 128 (batch, window) pairs: 16 per NeuronCore x 8 cores.
Each core runs the full LN + QKV projection + 4-head attention + output
projection + view-mean + skip-add for its pairs.

Device-side layout strategy (per (b,l) pair, Q=384 tokens, D=HD=128):
  - token-major LN (bn_stats/bn_aggr, rsqrt via ln/exp on ACT)
  - PE-transpose xhat -> feature-major xhatT
  - q,k projections feature-major (out [HD,384]); v projection token-major
  - scoresT[k,q] per head via 32-row tile-packed matmuls (4 heads concurrent)
  - exp on ACT straight out of PSUM (no max-subtract; |scores| ~ 8)
  - AV matmul with a fused "sixes" column producing 6*Z (softmax denom * 6,
    folding the 1/6 view-mean) as an extra output row; M=33, col-tiled 2/bank
  - softmax normalization deferred: av * (1/(6Z)) with 1/(6Z) broadcast
    across partitions by a tiny block-indicator matmul on the PE
  - view-mean as a strided reduce, output projection on [HD,64], skip add
Biases: LN gamma, q-scale, lnb/b biases are folded on the host (q-side bias
is exactly softmax-invariant; its k-dependent cross term is emitted as extra
accumulate matmuls only when nonzero).
"""

import numpy as np
from contextlib import ExitStack

import concourse.bass as bass
import concourse.tile as tile
from concourse import bacc, mybir
from concourse.bass_utils import run_bass_kernel_spmd

# Problem dims (hardcoded per contest rules).
B, N, X, Y, W1, W2, D = 2, 6, 8, 8, 8, 8, 128
H, DH = 4, 32
HD = H * DH
L = X * Y                  # 64 windows
Q = N * W1 * W2            # 384 tokens per window
BL = B * L                 # 128 (b,l) pairs
NCORES = 8
PER_CORE = BL // NCORES    # 16
NW = W1 * W2               # 64
EPS = 1e-5
F32 = mybir.dt.float32

_COMPILED = {}
USE_FP32R = True


def _emit(nc, with_qbias_cross):
    f32 = F32
    din = {}
    for t in ("xq", "xk", "xv"):
        din[t] = nc.dram_tensor(t, [PER_CORE, Q, D], f32, kind="ExternalInput").ap()
    skip_d = nc.dram_tensor("skipT", [PER_CORE, D, NW], f32, kind="ExternalInput").ap()
    _wdt = mybir.dt.float32r if USE_FP32R else f32
    wq_d = nc.dram_tensor("wq", [D, HD], _wdt, kind="ExternalInput").ap()
    wk_d = nc.dram_tensor("wk", [D, HD], _wdt, kind="ExternalInput").ap()
    wv_d = nc.dram_tensor("wv", [D, HD], _wdt, kind="ExternalInput").ap()
    wpa_d = nc.dram_tensor("wpA", [128, D], f32, kind="ExternalInput").ap()
    wpb_d = nc.dram_tensor("wpB", [128, D], f32, kind="ExternalInput").ap()
    bk_d = nc.dram_tensor("bk", [HD, 1], f32, kind="ExternalInput").ap()
    ea_d = nc.dram_tensor("EbA", [H, 128], f32, kind="ExternalInput").ap()
    eb_d = nc.dram_tensor("EbB", [H, 128], f32, kind="ExternalInput").ap()
    id_d = nc.dram_tensor("ident", [128, 128], f32, kind="ExternalInput").ap()
    # feature-major q-bias cross row (c = bqp^T kpT per head), built on device
    # from U = wk' @ diag-blocks... passed as [D, H] projection matrix instead.
    u_d = nc.dram_tensor("U", [D, H], f32, kind="ExternalInput").ap()
    g_d = nc.dram_tensor("gam", [H, 1], f32, kind="ExternalInput").ap()
    out_d = nc.dram_tensor("out", [PER_CORE, D, NW], f32, kind="ExternalOutput").ap()

    if USE_FP32R:
        r32 = lambda ap: ap.bitcast(mybir.dt.float32r)
        fr = mybir.dt.float32r
    else:
        r32 = lambda ap: ap
        fr = f32
    Exp = mybir.ActivationFunctionType.Exp
    Ln = mybir.ActivationFunctionType.Ln
    Alu = mybir.AluOpType

    with tile.TileContext(nc) as tc, ExitStack() as ctx:
        const = ctx.enter_context(tc.tile_pool(name="const", bufs=1))
        sb = ctx.enter_context(tc.tile_pool(name="sb", bufs=4))
        attp = ctx.enter_context(tc.tile_pool(name="attp", bufs=8))
        # PSUM: pa(2) + sc(4) + av(2) = 8 banks exactly
        pa = ctx.enter_context(tc.tile_pool(name="pa", bufs=2, space="PSUM"))
        scp = ctx.enter_context(tc.tile_pool(name="scp", bufs=2, space="PSUM"))
        avp = ctx.enter_context(tc.tile_pool(name="avp", bufs=1, space="PSUM"))
        dramp = ctx.enter_context(tc.tile_pool(name="dramp", bufs=2, space="DRAM"))

        def cload(name, ap_, shape, dt_=f32):
            t = const.tile(shape, dt_, tag=name, name=name)
            nc.sync.dma_start(t[:], ap_[:])
            return t

        wq_sb = cload("wq", wq_d, [D, HD], fr)
        wk_sb = cload("wk", wk_d, [D, HD], fr)
        wv_sb = cload("wv", wv_d, [D, HD], fr)
        wpa_sb = cload("wpA", wpa_d, [128, D])
        wpb_sb = cload("wpB", wpb_d, [128, D])
        bk_sb = cload("bk", bk_d, [HD, 1])
        ea_sb = cload("EbA", ea_d, [H, 128])
        eb_sb = cload("EbB", eb_d, [H, 128])
        id_sb = cload("ident", id_d, [128, 128])
        eps_sb = const.tile([128, 1], f32, tag="eps", name="eps")
        nc.vector.memset(eps_sb[:], EPS)
        if with_qbias_cross:
            u_sb = cload("U", u_d, [D, H])
            g_sb = cload("gam", g_d, [H, 1])
            ones_sb = const.tile([1, Q], fr, tag="ones")
            nc.vector.memset(ones_sb[:], 1.0)

        for bl in range(PER_CORE):
            # ---- load token-major x tiles [128 tok, 3 chunks, 128 D]
            x_sb = {}
            for t in ("xq", "xk", "xv"):
                x_sb[t] = sb.tile([128, 3, D], f32, tag=t, name=t)
                nc.sync.dma_start(
                    x_sb[t][:], din[t][bl].rearrange("(c p) d -> p c d", p=128)
                )
            skip_sb = sb.tile([D, NW], f32, tag="skip")
            nc.scalar.dma_start(skip_sb[:], skip_d[bl])

            # ---- LN stats (token-major): mean/var per token, r=1/sqrt(var+eps)
            st = sb.tile([128, 3, 3, 2], f32, tag="st")   # [tok, chunk, tensor, (mu,var)]
            for ti, t in enumerate(("xq", "xk", "xv")):
                bn6 = sb.tile([128, 3, 6], f32, tag="bn6")
                for c in range(3):
                    nc.vector.bn_stats(bn6[:, c, :], x_sb[t][:, c, :])
                    nc.vector.bn_aggr(st[:, c, ti, :], bn6[:, c, :])
            # r9 = rsqrt(var+eps) via bit-trick seed + 2 Newton steps
            # (keeps ACT's table set pinned to exp_and_others: no reloads)
            i32 = mybir.dt.int32
            v9 = sb.tile([128, 3, 3], f32, tag="v9")
            nc.gpsimd.tensor_scalar_add(v9[:], st[:, :, :, 1], EPS)
            r9 = sb.tile([128, 3, 3], f32, tag="r9")
            nc.vector.tensor_scalar(
                r9[:].bitcast(i32), v9[:].bitcast(i32), 1, None,
                op0=Alu.arith_shift_right,
            )
            nc.vector.tensor_scalar(
                r9[:].bitcast(i32), r9[:].bitcast(i32), -1, 0x5F3759DF,
                op0=Alu.mult, op1=Alu.add,
            )
            t9 = sb.tile([128, 3, 3], f32, tag="t9")
            for _ in range(2):
                nc.gpsimd.tensor_tensor(t9[:], r9[:], r9[:], op=Alu.mult)
                nc.gpsimd.tensor_tensor(t9[:], t9[:], v9[:], op=Alu.mult)
                nc.gpsimd.tensor_scalar(
                    t9[:], t9[:], -0.5, 1.5, op0=Alu.mult, op1=Alu.add
                )
                nc.gpsimd.tensor_tensor(r9[:], r9[:], t9[:], op=Alu.mult)

            # ---- xhat = (x - mu) * r (gpsimd), then PE-transpose to xhatT
            xhT_sb = {}
            for ti, t in enumerate(("xq", "xk", "xv")):
                xh = sb.tile([128, 3, D], f32, tag=f"xh{t}")
                for c in range(3):
                    nc.gpsimd.tensor_scalar(
                        xh[:, c, :], x_sb[t][:, c, :],
                        st[:, c, ti, 0:1], r9[:, c, ti : ti + 1],
                        op0=Alu.subtract, op1=Alu.mult,
                    )
                xhT_ps = pa.tile([128, 512], f32, tag="pa")
                for c in range(3):
                    nc.tensor.transpose(
                        xhT_ps[:, 128 * c : 128 * (c + 1)], xh[:, c, :], id_sb[:]
                    )
                xhT_sb[t] = sb.tile([D, Q], fr, tag=f"xhT{t}", name=f"xhT{t}")
                if t == "xq":
                    nc.vector.tensor_copy(xhT_sb[t][:], xhT_ps[:, 0:Q])
                else:
                    nc.scalar.copy(xhT_sb[t][:], xhT_ps[:, 0:Q])

            # ---- projections: q,k feature-major [HD, 384]; v token-major
            qpT_sb = sb.tile([HD, Q], fr, tag="qpT")
            kpT_sb = sb.tile([HD, Q], fr, tag="kpT")
            for wsb, xsb, osb, bias in (
                (wq_sb, xhT_sb["xq"], qpT_sb, None),
                (wk_sb, xhT_sb["xk"], kpT_sb, bk_sb),
            ):
                pp = pa.tile([128, 512], f32, tag="pa")
                nc.tensor.matmul(pp[:, 0:Q], r32(wsb[:]), r32(xsb[:]))
                if bias is None:
                    nc.scalar.copy(osb[:], pp[:, 0:Q])
                else:
                    nc.vector.tensor_scalar(
                        osb[:], pp[:, 0:Q], bias[:], None, op0=Alu.add
                    )
            if with_qbias_cross:
                # kaugT[h, k] = xhat_k . U_h + gamma_h, feature-major row per head
                ka_ps = pa.tile([128, 512], f32, tag="pa")
                nc.tensor.matmul(ka_ps[0:H, 0:Q], u_sb[:], xhT_sb["xk"][:])
                ka_sb = sb.tile([H, Q], fr, tag="ka")
                nc.vector.tensor_scalar(
                    ka_sb[:], ka_ps[0:H, 0:Q], g_sb[0:H, :], None, op0=Alu.add
                )

            vp_ps = pa.tile([128, 512], f32, tag="pa")
            for c in range(3):
                nc.tensor.matmul(
                    vp_ps[:, 128 * c : 128 * (c + 1)],
                    xhT_sb["xv"][:, 128 * c : 128 * (c + 1)], wv_sb[:],
                )
            # vpe: per chunk, per head: [v_h (32 cols) | 6.0] -> [128, 3, 132]
            vpe = sb.tile([128, 3, H * 2 * DH], mybir.dt.bfloat16, tag="vpe")
            for c in range(3):
                vv = vpe[:, c, :].rearrange("p (h w) -> p h w", w=2 * DH)
                nc.vector.tensor_copy(
                    vv[:, :, 0:DH],
                    vp_ps[:, 128 * c : 128 * (c + 1)].rearrange(
                        "p (h w) -> p h w", w=DH
                    ),
                )
                nc.gpsimd.memset(vv[:, :, DH : 2 * DH], 6.0)

            # ---- attention: scoresT + exp per k-chunk (2-head groups,
            # double-buffered so scores(c+1) overlaps exp(c)), then AV
            av_ps = avp.tile([128, 1024], f32, tag="av")
            atts = {}
            for c in range(3):
                for g in range(2):           # head pair group
                    sc_ps = scp.tile([128, 1024], f32, tag="sc", name=f"sc{c}{g}")
                    for hh in range(2):
                        h = 2 * g + hh
                        nc.tensor.matmul(
                            sc_ps[:, 512 * hh : 512 * hh + Q],
                            r32(kpT_sb[32 * h : 32 * (h + 1), 128 * c : 128 * (c + 1)]),
                            r32(qpT_sb[32 * h : 32 * (h + 1), :]),
                            tile_position=(32 * h, 0),
                            start=True, stop=not with_qbias_cross,
                        )
                        if with_qbias_cross:
                            nc.tensor.matmul(
                                sc_ps[:, 512 * hh : 512 * hh + Q],
                                r32(ka_sb[h : h + 1, 128 * c : 128 * (c + 1)]),
                                r32(ones_sb[:]),
                                start=False, stop=True,
                            )
                    att = attp.tile(
                        [128, 2, Q], mybir.dt.bfloat16, tag="att", name=f"att{c}{g}"
                    )
                    nc.scalar.activation(
                        att[:],
                        sc_ps[:].rearrange("p (h q) -> p h q", q=512)[:, :, 0:Q],
                        Exp,
                    )
                    atts[(c, g)] = att
            for h in range(H):
                po, bo = 64 * (h % 2), 512 * (h // 2)
                for c in range(3):
                    nc.tensor.matmul(
                        av_ps[po : po + 2 * DH, bo : bo + Q],
                        vpe[:, c, 2 * DH * h : 2 * DH * (h + 1)],
                        atts[(c, h // 2)][:, h % 2, :],
                        tile_position=(0, po),
                        start=(c == 0), stop=(c == 2),
                    )

            # ---- normalize by 1/(6Z), mean over views, project, add skip
            # av_ps native layout: bank b (cols 512b), head-pair member m at
            # partitions 64m:64m+32, its 6Z row at partition 64m+32.
            zi_sb = sb.tile([128, 2, Q], f32, tag="zi_sb")
            for p2 in range(2):
                po = 32 + 64 * p2
                nc.vector.reciprocal(
                    zi_sb[po : po + DH],
                    av_ps[po : po + DH].rearrange("p (b q) -> p b q", q=512)[:, :, 0:Q],
                )
            zbb = sb.tile([128, 2, Q], f32, tag="zbb")
            for p2 in range(2):
                nc.scalar.dma_start(
                    zbb[64 * p2 : 64 * p2 + DH, :, :],
                    zi_sb[32 + 64 * p2 : 64 * p2 + 2 * DH, :, :],
                )
            avn = sb.tile([128, 2, Q], f32, tag="avn")
            nc.gpsimd.memset(avn[32:64, :, :], 0.0)
            nc.gpsimd.memset(avn[96:128, :, :], 0.0)
            for po in (0, 64):
                nc.vector.tensor_tensor(
                    avn[po : po + DH, :, :],
                    av_ps[po : po + DH].rearrange("p (b q) -> p b q", q=512)[:, :, 0:Q],
                    zbb[po : po + DH, :, :],
                    op=Alu.mult,
                )
            avm = sb.tile([128, 2, NW], f32, tag="avm")
            nc.vector.reduce_sum(
                avm[:], avn[:].rearrange("p b (n w) -> p b w n", n=N),
                axis=mybir.AxisListType.X,
            )
            z_ps = pa.tile([128, 512], f32, tag="pa", name="z_ps")
            nc.tensor.matmul(z_ps[:, 0:NW], wpa_sb[:], avm[:, 0, :], start=True, stop=False)
            nc.tensor.matmul(z_ps[:, 0:NW], wpb_sb[:], avm[:, 1, :], start=False, stop=True)
            zo = sb.tile([D, NW], f32, tag="zo")
            nc.vector.tensor_tensor(zo[:], z_ps[:, 0:NW], skip_sb[:], op=Alu.add)
            nc.scalar.dma_start(out_d[bl], zo[:])


def _build(with_qbias_cross):
    key = (bool(with_qbias_cross), USE_FP32R)
    if key in _COMPILED:
        return _COMPILED[key]
    nc = bacc.Bacc("TRN2", target_bir_lowering=False, debug=False)
    _emit(nc, bool(with_qbias_cross))
    nc.compile()
    _COMPILED[key] = nc
    return nc


def _prep_host(inputs):
    q, k, v, skip = inputs["q"], inputs["k"], inputs["v"], inputs["skip"]
    scale = np.float32(DH ** -0.5)
    fold = lambda t: np.ascontiguousarray(
        t.transpose(0, 2, 3, 1, 4, 5, 6).reshape(BL, Q, D)
    )
    xq, xk, xv = fold(q), fold(k), fold(v)
    wq = (inputs["lnq_g"][:, None] * inputs["wq"] * scale).astype(np.float32)
    wk = (inputs["lnk_g"][:, None] * inputs["wk"]).astype(np.float32)
    wv = (inputs["lnv_g"][:, None] * inputs["wv"]).astype(np.float32)
    wp = inputs["wp"].astype(np.float32)
    bkp = (inputs["lnk_b"] @ inputs["wk"] + inputs["bk"]).astype(np.float32)
    bqp = ((inputs["lnq_b"] @ inputs["wq"] + inputs["bq"]) * scale).astype(np.float32)
    bvp = (inputs["lnv_b"] @ inputs["wv"] + inputs["bv"]).astype(np.float32)
    skipT = np.ascontiguousarray(
        (skip.reshape(BL, NW, D) + inputs["bp"] + bvp @ wp).transpose(0, 2, 1)
    ).astype(np.float32)
    # q-side bias: softmax-invariant part drops; k-dependent cross term needs
    # U[:, h] = wk'_hblock @ bqp_hblock and gamma_h = bk'_h . bqp_h
    U = np.zeros((D, H), np.float32)
    gam = np.zeros((H, 1), np.float32)
    for h in range(H):
        s = slice(h * DH, (h + 1) * DH)
        U[:, h] = wk[:, s] @ bqp[s]
        gam[h, 0] = bkp[s] @ bqp[s]
    with_cross = bool(np.abs(bqp).max() > 0)
    # native AV layout remap: bank A heads 0,1; bank B heads 2,3;
    # head pair member m at partitions 64m..64m+32 (Z row at 64m+32)
    wpA = np.zeros((128, D), np.float32)
    wpB = np.zeros((128, D), np.float32)
    EbA = np.zeros((H, 128), np.float32)
    EbB = np.zeros((H, 128), np.float32)
    for m in range(2):
        po = 64 * m
        wpA[po : po + DH] = wp[m * DH : (m + 1) * DH]
        wpB[po : po + DH] = wp[(2 + m) * DH : (3 + m) * DH]
        # zrow row r = 2*p2 + b holds (6Z) of head h = 2b + p2; here p2 = m.
        EbA[2 * m + 0, po : po + DH] = 1.0   # bank A (b=0): heads 0,1
        EbB[2 * m + 1, po : po + DH] = 1.0   # bank B (b=1): heads 2,3
    ident = np.eye(128, dtype=np.float32)
    consts = dict(
        wq=wq, wk=wk, wv=wv, wpA=wpA, wpB=wpB, bk=bkp.reshape(HD, 1),
        EbA=EbA, EbB=EbB, ident=ident, U=U, gam=gam,
    )
    in_maps = []
    for c in range(NCORES):
        s = slice(c * PER_CORE, (c + 1) * PER_CORE)
        m = dict(
            xq=np.ascontiguousarray(xq[s]),
            xk=np.ascontiguousarray(xk[s]),
            xv=np.ascontiguousarray(xv[s]),
            skipT=np.ascontiguousarray(skipT[s]),
        )
        m.update({k_: v_.copy() for k_, v_ in consts.items()})
        in_maps.append(m)
    return in_maps, with_cross


def kernel(**inputs):
    inputs = {k: np.asarray(v, dtype=np.float32) for k, v in inputs.items()}
    in_maps, with_cross = _prep_host(inputs)
    nc = _build(with_cross)
    res = run_bass_kernel_spmd(nc, in_maps, list(range(NCORES)))
    zT = np.concatenate([r["out"] for r in res.results], axis=0)  # [BL, D, 64]
    z = zT.transpose(0, 2, 1).reshape(B, X, Y, W1, W2, D)
    return np.ascontiguousarray(z)
